# revision 2
# baseline (speedup 1.0000x reference)
"""Trainium2 Bass kernel v2 for nn_DirectionalScan (2D directional diagonal-SSM + projection).

Math per direction (scan over h, scan over w):
    y[t] = sum_n Cm*Bm * sum_{u<=t} A^(t-u) x[u]  + D_skip*x[t]
then out = (y_h + y_v) @ Wp.T + b_proj.

v2 changes vs baseline:
  - host pre-packs x into feature-major (j,e)x(o,c,s) layout -> no on-chip input
    transposes, no x_perm reorder, all input DMAs fully contiguous
  - weights pre-packed into final SBUF layout (single-descriptor DMAs)
  - outputs written fp16 in SBUF-native layout; host unpermutes (free)
  - chunk-state recurrence batched: one PSUM->SBUF staging copy per q-group,
    then 6 full-width DVE ops per direction
  - the two directions' phases are interleaved to keep the PE busy

Sharding: 8 cores; core k handles batch b=k//2 and half=k%2:
  vertical  (scan over w): sequences (b, h in [32*half, 32*half+32))
  horizontal(scan over h): sequences (b, w in [32*half, 32*half+32))
"""
from contextlib import ExitStack

import numpy as np

import concourse.bass as bass
import concourse.bacc as bacc
import concourse.tile as tile
from concourse import mybir
from concourse.bass_utils import run_bass_kernel_spmd
from concourse.masks import make_identity

F32 = mybir.dt.float32
F16 = mybir.dt.float16
NP_CDT = np.float16
B, H, W, D, N = 4, 64, 64, 512, 8
L, Q, C, SEQ = 64, 16, 4, 32   # seq len, chunk size, n chunks, seqs/core/direction
NOCT = 64                      # octets of 8 channels
NG = 32                        # 2-octet groups


# ----------------------------------------------------------------------------
# host-side packing
# ----------------------------------------------------------------------------

def _precompute_weights(A, Bm, Cm, D_skip, Wp):
    A64, B64, C64 = A.astype(np.float64), Bm.astype(np.float64), Cm.astype(np.float64)
    CB = C64 * B64                                   # [D, N]
    Apow = np.stack([A64 ** t for t in range(Q + 1)])  # [Q+1, D, N]
    Kconv = np.einsum("dn,tdn->dt", CB, Apow)        # [D, Q+1]
    T = np.zeros((D, Q, Q))
    for i in range(Q):
        for j in range(i + 1):
            T[:, i, j] = Kconv[:, i - j]
    T += np.eye(Q)[None] * D_skip.astype(np.float64)[:, None, None]

    # W_T: rows (j16,e8), cols (i16,e8) per octet; block-diag in e
    W_T = np.zeros((NOCT, 128, 128))
    W_P = np.zeros((NOCT, 128, 64))
    for o in range(NOCT):
        for d8 in range(8):
            d = o * 8 + d8
            for j in range(Q):
                W_T[o, j * 8 + d8, d8::8] = T[d, :, j]
                W_P[o, j * 8 + d8, d8 * 8:d8 * 8 + 8] = Apow[Q - 1 - j, d]
    W_CBA = np.zeros((NG, 128, 256))
    for g in range(NG):
        for o2 in range(2):
            for d8 in range(8):
                d = g * 16 + o2 * 8 + d8
                for n in range(N):
                    row = o2 * 64 + d8 * 8 + n
                    W_CBA[g, row, o2 * 128 + d8:o2 * 128 + 128:8] = (
                        CB[d, n] * Apow[1:Q + 1, d, n]
                    )
    A16 = np.zeros((128, NG))
    for g in range(NG):
        for o2 in range(2):
            for d8 in range(8):
                d = g * 16 + o2 * 8 + d8
                A16[o2 * 64 + d8 * 8:o2 * 64 + d8 * 8 + 8, g] = Apow[Q, d]
    A16 = np.repeat(A16, SEQ, axis=1)  # [128, (g32, s32)]
    WPT = Wp.astype(np.float64).T.reshape(4, 128, 512)
    # pack into SBUF-native [128, ...] layouts
    w_t = np.ascontiguousarray(W_T.transpose(1, 0, 2).reshape(128, NOCT * 128))
    w_p = np.ascontiguousarray(W_P.transpose(1, 0, 2).reshape(128, NOCT * 64))
    w_cba = np.ascontiguousarray(W_CBA.transpose(1, 0, 2).reshape(128, NG * 256))
    wpt = np.ascontiguousarray(WPT.transpose(1, 0, 2).reshape(128, 4 * 512))
    return (w_t.astype(NP_CDT), w_p.astype(NP_CDT), w_cba.astype(NP_CDT),
            A16.astype(NP_CDT), wpt.astype(NP_CDT))


def _pack_x(xg_b, hsl):
    """Feature-major packs for one core: xt[(j,e), (o,c,s)]."""
    # vertical: sequences s = rows h in hsl; positions w = c*16+j
    xv = xg_b[hsl]                                # [32s, 64w, 512d]
    xv = xv.reshape(SEQ, C, Q, NOCT, 8)           # [s, c, j, o, e]
    xv = xv.transpose(2, 4, 3, 1, 0)              # [j, e, o, c, s]
    xv = np.ascontiguousarray(xv).reshape(128, NOCT * C * SEQ)
    # horizontal: sequences s = cols w in hsl; positions h = c*16+j
    xh = xg_b[:, hsl]                             # [64h, 32s, 512d]
    xh = xh.reshape(C, Q, SEQ, NOCT, 8)           # [c, j, s, o, e]
    xh = xh.transpose(1, 4, 3, 0, 2)              # [j, e, o, c, s]
    xh = np.ascontiguousarray(xh).reshape(128, NOCT * C * SEQ)
    return xv.astype(NP_CDT), xh.astype(NP_CDT)


# ----------------------------------------------------------------------------
# device program
# ----------------------------------------------------------------------------

def _emit_g(tc, pools, consts, xt, tag):
    """Chunk-increment matmuls + staging copy; returns (g_sb, s4) tiles."""
    nc = tc.nc
    (s_pool, g_pool, y_pool, yt_pool, out_pool, psG, psyw, psyt, psout) = pools
    w_t_sb, w_p_sb, w_cba_sb, a16_sb, wpt_sb, ident = consts

    g_sb = g_pool.tile([128, 8 * 512], F16, tag="g", name=f"g_{tag}")
    for q in range(8):  # 8 octets per q-group
        ps_g = psG.tile([128, 512], F32, tag="ps_g")
        for k in range(8):
            o = q * 8 + k
            half = (o % 2) * 64
            col = (k // 2) * 128
            nc.tensor.matmul(
                ps_g[half:half + 64, col:col + 128],
                w_p_sb[:, o * 64:o * 64 + 64], xt[:, o * 128:(o + 1) * 128],
                start=True, stop=True, skip_group_check=True,
                tile_position=(0, half))
        nc.any.tensor_copy(g_sb[:, q * 512:(q + 1) * 512], ps_g[:])
    # batched chunk-state recurrence: s_c = a16*s_{c-1} + g_{c-1}, s_0 = 0
    s4 = s_pool.tile([128, 8 * 512], F16, tag="s", name=f"s_{tag}")
    sv = s4[:].rearrange("p (q g c s) -> p (q g) c s", q=8, g=4, c=C)
    gv = g_sb[:].rearrange("p (q g c s) -> p (q g) c s", q=8, g=4, c=C)
    a16v = a16_sb[:].rearrange("p (g s) -> p g s", g=NG)
    nc.gpsimd.memset(sv[:, :, 0, :], 0.0)
    nc.any.tensor_copy(sv[:, :, 1, :], gv[:, :, 0, :])
    for cc in (2, 3):
        nc.any.tensor_mul(sv[:, :, cc, :], sv[:, :, cc - 1, :], a16v)
        nc.any.tensor_add(sv[:, :, cc, :], sv[:, :, cc, :], gv[:, :, cc - 1, :])
    return s4


def _emit_tcba(tc, pools, consts, xt, s4, tag):
    """Intra-chunk Toeplitz + inter-chunk matmuls; returns y_sb (i-major)."""
    nc = tc.nc
    (s_pool, g_pool, y_pool, yt_pool, out_pool, psG, psyw, psyt, psout) = pools
    w_t_sb, w_p_sb, w_cba_sb, a16_sb, wpt_sb, ident = consts

    y_sb = y_pool.tile([128, Q * D], F16, tag="y", name=f"y_{tag}")
    for og in range(16):
        ps_yw = psyw.tile([128, 512], F32, tag="ps_yw")
        for oo in range(4):
            o = og * 4 + oo
            nc.tensor.matmul(ps_yw[:, oo * 128:oo * 128 + 128],
                             xt[:, o * 128:(o + 1) * 128],
                             w_t_sb[:, o * 128:(o + 1) * 128],
                             start=(oo == 0), stop=False, skip_group_check=True)
        for gg in range(2):
            g = og * 2 + gg
            nc.tensor.matmul(ps_yw[:, gg * 256:gg * 256 + 256],
                             s4[:, g * 128:g * 128 + 128],
                             w_cba_sb[:, g * 256:g * 256 + 256],
                             start=False, stop=(gg == 1), skip_group_check=True)
        # scatter into y_sb layout (i16, d512-contig); ps_yw cols are (o4,i16,e8)
        y_dst = y_sb[:].rearrange("p (i og o e) -> p i og o e",
                                  i=Q, og=16, o=4, e=8)[:, :, og]
        ps_src = ps_yw[:].rearrange("p (o i e) -> p i o e", o=4, i=Q, e=8)
        nc.any.tensor_copy(y_dst, ps_src)
    return y_sb


def _emit_proj(tc, pools, consts, y_sb, z_view, tag, out_eng):
    """Per-i transpose + projection matmuls + output DMA."""
    nc = tc.nc
    (s_pool, g_pool, y_pool, yt_pool, out_pool, psG, psyw, psyt, psout) = pools
    w_t_sb, w_p_sb, w_cba_sb, a16_sb, wpt_sb, ident = consts

    def emit_trans(i):
        """PE-transpose y[:, i-block] to feature-major and evacuate."""
        ps_yt = psyt.tile([128, 512], F16, tag="ps_yt")
        for dc in range(4):
            nc.tensor.transpose(
                ps_yt[:, dc * 128:(dc + 1) * 128],
                y_sb[:, i * 512 + dc * 128:i * 512 + (dc + 1) * 128], ident)
        yt = yt_pool.tile([128, 512], F16, tag="yt")
        nc.any.tensor_copy(yt[:], ps_yt[:])
        return yt

    # software-pipelined: transpose(i+2) is emitted before proj(i) so the
    # in-order PE queue never waits on a yt evacuation
    yts = {}
    yts[0] = emit_trans(0)
    yts[1] = emit_trans(1)
    out_sbs = {}
    for i in range(16):
        if i + 2 < 16:
            yts[i + 2] = emit_trans(i + 2)
        iq, ii = divmod(i, 4)
        if ii == 0:
            out_sbs[iq] = out_pool.tile([128, 4 * 512], F16, tag="osb", name=f"osb_{tag}{iq}")
        yt = yts.pop(i)
        ps_o = psout.tile([128, 512], F32, tag="ps_o")
        for dc in range(4):
            nc.tensor.matmul(ps_o[:], yt[:, dc * 128:(dc + 1) * 128],
                             wpt_sb[:, dc * 512:(dc + 1) * 512],
                             start=(dc == 0), stop=(dc == 3))
        nc.any.tensor_copy(out_sbs[iq][:, ii * 512:(ii + 1) * 512], ps_o[:])
        if ii == 3:
            out_eng.dma_start(z_view[:, iq * 2048:(iq + 1) * 2048],
                              out_sbs.pop(iq)[:])


def _kernel_body(ctx, tc, aps):
    nc = tc.nc
    const_pool = ctx.enter_context(tc.tile_pool(name="consts", bufs=1))
    xt_pool = ctx.enter_context(tc.tile_pool(name="xt", bufs=2))
    s_pool = ctx.enter_context(tc.tile_pool(name="s", bufs=2))
    g_pool = ctx.enter_context(tc.tile_pool(name="g", bufs=2))
    y_pool = ctx.enter_context(tc.tile_pool(name="y", bufs=2))
    yt_pool = ctx.enter_context(tc.tile_pool(name="yt", bufs=4))
    out_pool = ctx.enter_context(tc.tile_pool(name="osb", bufs=3))
    psG = ctx.enter_context(tc.tile_pool(name="psG", bufs=2, space="PSUM"))
    psyw = ctx.enter_context(tc.tile_pool(name="psyw", bufs=2, space="PSUM"))
    psyt = ctx.enter_context(tc.tile_pool(name="psyt", bufs=2, space="PSUM"))
    psout = ctx.enter_context(tc.tile_pool(name="psout", bufs=2, space="PSUM"))
    pools = (s_pool, g_pool, y_pool, yt_pool, out_pool, psG, psyw, psyt, psout)

    w_t_sb = const_pool.tile([128, NOCT * 128], F16, name="w_t_sb")
    w_p_sb = const_pool.tile([128, NOCT * 64], F16, name="w_p_sb")
    w_cba_sb = const_pool.tile([128, NG * 256], F16, name="w_cba_sb")
    a16_sb = const_pool.tile([128, NG * SEQ], F16, name="a16_sb")
    wpt_sb = const_pool.tile([128, 4 * 512], F16, name="wpt_sb")
    ident = const_pool.tile([128, 128], F16, name="ident")
    xt_v = xt_pool.tile([128, NOCT * 128], F16, tag="xtv", name="xt_v")
    xt_h = xt_pool.tile([128, NOCT * 128], F16, tag="xth", name="xt_h")

    # input DMAs in need order, big chunks (>=8KB per partition line where
    # possible).  sync: xv then w_t; scalar: w_p/a16/w_cba/wpt; gpsimd: xh.
    # Outputs later ride sync (zh) / scalar (zv) once inputs have drained.
    nc.scalar.dma_start(w_p_sb[:, :2048], aps["w_p"][:, :2048])
    nc.sync.dma_start(xt_v[:, :4096], aps["xv"][:, :4096])
    nc.scalar.dma_start(w_p_sb[:, 2048:], aps["w_p"][:, 2048:])
    nc.sync.dma_start(xt_v[:, 4096:], aps["xv"][:, 4096:])
    nc.scalar.dma_start(a16_sb[:], aps["a16"])
    nc.gpsimd.dma_start(xt_h[:, :4096], aps["xh"][:, :4096])
    nc.gpsimd.dma_start(xt_h[:, 4096:], aps["xh"][:, 4096:])
    nc.sync.dma_start(w_t_sb[:, :4096], aps["w_t"][:, :4096])
    nc.scalar.dma_start(w_cba_sb[:, :4096], aps["w_cba"][:, :4096])
    nc.sync.dma_start(w_t_sb[:, 4096:], aps["w_t"][:, 4096:])
    nc.scalar.dma_start(w_cba_sb[:, 4096:], aps["w_cba"][:, 4096:])
    nc.scalar.dma_start(wpt_sb[:], aps["wpt"])
    make_identity(nc, ident[:])
    consts = (w_t_sb[:], w_p_sb[:], w_cba_sb[:], a16_sb[:], wpt_sb[:], ident[:])

    # interleaved: G-h bridges the TCBA-v -> proj-v evacuation boundary
    s4_v = _emit_g(tc, pools, consts, xt_v[:], "v")
    y_v = _emit_tcba(tc, pools, consts, xt_v[:], s4_v[:], "v")
    s4_h = _emit_g(tc, pools, consts, xt_h[:], "h")
    _emit_proj(tc, pools, consts, y_v[:], aps["zv"], "v", nc.scalar)
    y_h = _emit_tcba(tc, pools, consts, xt_h[:], s4_h[:], "h")
    _emit_proj(tc, pools, consts, y_h[:], aps["zh"], "h", nc.sync)


def build_program(n_cores=8):
    nc = bacc.Bacc("TRN2", target_bir_lowering=False, debug=False,
                   enable_asserts=False, num_devices=n_cores)
    aps = {
        "xv": nc.dram_tensor("xv", [128, NOCT * 128], F16, kind="ExternalInput").ap(),
        "xh": nc.dram_tensor("xh", [128, NOCT * 128], F16, kind="ExternalInput").ap(),
        "w_t": nc.dram_tensor("w_t", [128, NOCT * 128], F16, kind="ExternalInput").ap(),
        "w_p": nc.dram_tensor("w_p", [128, NOCT * 64], F16, kind="ExternalInput").ap(),
        "w_cba": nc.dram_tensor("w_cba", [128, NG * 256], F16, kind="ExternalInput").ap(),
        "a16": nc.dram_tensor("a16", [128, NG * SEQ], F16, kind="ExternalInput").ap(),
        "wpt": nc.dram_tensor("wpt", [128, 4 * 512], F16, kind="ExternalInput").ap(),
        "zv": nc.dram_tensor("zv", [128, Q * D], F16, kind="ExternalOutput").ap(),
        "zh": nc.dram_tensor("zh", [128, Q * D], F16, kind="ExternalOutput").ap(),
    }
    with tile.TileContext(nc) as tc:
        with ExitStack() as ctx:
            _kernel_body(ctx, tc, aps)
    nc.compile()
    return nc


_PROGRAM = None


def _get_program():
    global _PROGRAM
    if _PROGRAM is None:
        _PROGRAM = build_program()
    return _PROGRAM


def make_in_maps(x, A, Bm, Cm, D_skip, Wp):
    w_t, w_p, w_cba, a16, wpt = _precompute_weights(A, Bm, Cm, D_skip, Wp)
    xg = np.ascontiguousarray(x, dtype=np.float32).reshape(B, H, W, D)
    in_maps = []
    for k in range(8):
        b, half = k // 2, k % 2
        xv, xh = _pack_x(xg[b], slice(32 * half, 32 * half + 32))
        in_maps.append({
            "xv": xv, "xh": xh,
            "w_t": w_t, "w_p": w_p, "w_cba": w_cba, "a16": a16, "wpt": wpt,
        })
    return in_maps


def assemble_output(results, b_proj):
    out = np.zeros((B, H, W, D), np.float32)
    for k in range(8):
        b, half = k // 2, k % 2
        hsl = slice(32 * half, 32 * half + 32)
        zv = results[k]["zv"].astype(np.float32).reshape(C, SEQ, Q, D)
        # partition (c,s): sequence s = row h, position w = c*16+i
        out[b, hsl, :, :] += zv.transpose(1, 0, 2, 3).reshape(SEQ, L, D)
        zh = results[k]["zh"].astype(np.float32).reshape(C, SEQ, Q, D)
        # partition (c,s): sequence s = col w, position h = c*16+i
        out[b, :, hsl, :] += zh.transpose(0, 2, 1, 3).reshape(L, SEQ, D)
    out += np.asarray(b_proj, dtype=np.float32)
    return out.reshape(B, H * W, D)


def kernel(x, h, w, A, Bm, Cm, D_skip, Wp, b_proj, **_kw):
    nc = _get_program()
    in_maps = make_in_maps(np.asarray(x), np.asarray(A), np.asarray(Bm),
                           np.asarray(Cm), np.asarray(D_skip), np.asarray(Wp))
    res = run_bass_kernel_spmd(nc, in_maps, list(range(8)))
    return assemble_output(res.results, np.asarray(b_proj))
